# revision 28
# baseline (speedup 1.0000x reference)
"""GATv2 layer kernel for 8 Trainium2 NeuronCores.

Mathematical structure exploited: in this GATv2 variant the value vectors are
gathered at the *destination* node (Vv = node_feats[dest] @ W_v + b_v), so for
every destination node d the aggregation

    out[d] = sum_{e: dest_e = d} alpha_e * (node_feats[d] @ W_v + b_v)
           = (node_feats[d] @ W_v + b_v) * sum_e alpha_e
           = (node_feats[d] @ W_v + b_v) * [deg_in(d) > 0]

because the softmax weights alpha sum to exactly 1 within each destination
segment (and the sum is empty for isolated nodes). Q/K/edge_feats/a_w only
reweight terms inside a softmax that cancels entirely. Verified against the
reference: max relative error ~2.6e-7 (pure fp32 rounding).

Device computation per core c (nodes sharded 6272/core): the node block is
split into two column halves stacked on the two 64-partition halves of SBUF
so every DMA runs at full 128-partition width. Per 392-column chunk and half:
one weight-stationary matmul (out.T = W_v.T @ x.T), then one fused DVE
scalar_tensor_tensor (add bias per output feature, multiply by the presence
mask). The mask ships as a pre-broadcast bf16 plane; W and b ship duplicated
per partition half inside the packed x tensor. The presence bitmap (50 KB)
is folded on the host during input sharding.

Sync-wait discipline: this container's walrus build allows only ONE semaphore
wait per instruction and Tile's sem assignment is not transitive, so the
stream is arranged so every op needs at most one new wait: a PE warm-up
matmul observes the first load, PE/DVE observers absorb the second-segment
loads, Pool memsets observe the DVE results before each SWDGE store, and a
final SP NoOp chain observes all async completions so the kernel-tail drain
needs no waits of its own.
"""
import ml_dtypes
import numpy as np

import concourse.bass as bass
import concourse.mybir as mybir
import concourse.tile as tile
from concourse.bass_utils import run_bass_kernel_spmd
from concourse.tile_rust import add_dep_helper

V, E = 50000, 800000
D_IN, D_OUT = 64, 64
NCORES = 8
P = 128
SHARD = 6272                # nodes per core
HALF = SHARD // 2           # 3136 columns per partition-half
VPAD = SHARD * NCORES       # 50176
MM = 392                    # node columns per matmul instruction
BIG = MM * 4                # 1568-column fused round (= load/store segment)
XOFF = D_OUT + 1            # x columns start after packed W (64) + b (1)
SPLIT = BIG                 # segment boundary in node columns

_cache = {}


def _build():
    nc = bass.Bass()
    xt = nc.dram_tensor("xt", [P, XOFF + HALF], mybir.dt.float32, kind="ExternalInput")
    mb_d = nc.dram_tensor("mb", [P, HALF], mybir.dt.bfloat16, kind="ExternalInput")
    out_t = nc.dram_tensor("out_t", [P, HALF], mybir.dt.float32, kind="ExternalOutput")

    with tile.TileContext(nc) as tc:
        with (
            tc.tile_pool(name="const", bufs=1) as const,
            tc.tile_pool(name="po", bufs=1, space="PSUM") as po,
            tc.tile_pool(name="pd", bufs=1, space="PSUM") as pd,
        ):
            xt_sb = const.tile([P, XOFF + HALF], mybir.dt.float32)
            l1 = nc.sync.dma_start(out=xt_sb[:, : XOFF + SPLIT], in_=xt[:, : XOFF + SPLIT])
            mask_sb = const.tile([P, HALF], mybir.dt.bfloat16)
            lm1 = nc.sync.dma_start(out=mask_sb[:, :SPLIT], in_=mb_d[:, :SPLIT])
            l2 = nc.sync.dma_start(out=xt_sb[:, XOFF + SPLIT :], in_=xt[:, XOFF + SPLIT :])
            lm2 = nc.sync.dma_start(out=mask_sb[:, SPLIT:], in_=mb_d[:, SPLIT:])

            o_sb = const.tile([P, HALF], mybir.dt.float32)
            scratch = const.tile([1, 20], mybir.dt.float32)
            scratch2 = const.tile([1, 40], mybir.dt.float32)

            # PE warm-up observes load1; DVE observers for load1/mask1
            dummy = pd.tile([D_OUT, 1], mybir.dt.float32)
            mw = nc.tensor.matmul(dummy[:], lhsT=xt_sb[:D_OUT, :D_OUT], rhs=xt_sb[:D_OUT, 0:1], start=True, stop=True)
            add_dep_helper(mw.ins, l1.ins, True, "warm PE: observe load1")
            dvm0 = nc.vector.memset(scratch2[:, 0:1], 0.0)
            add_dep_helper(dvm0.ins, l1.ins, True, "DVE observes load1")
            dvm1 = nc.vector.memset(scratch2[:, 1:2], 0.0)
            add_dep_helper(dvm1.ins, lm1.ins, True, "DVE observes mask1")
            add_dep_helper(dvm1.ins, dvm0.ins, False, "DVE order")

            tts, mms, pool_obs, stores = [], [], [], []
            prev_pe = mw
            prev_dve = dvm1

            def emit_store(lo, hi, members):
                prev = pool_obs[-1] if pool_obs else None
                for k, t in members:
                    ob = nc.gpsimd.memset(scratch[:, k : k + 1], 0.0)
                    add_dep_helper(ob.ins, t.ins, True, "Pool observes TT")
                    if prev is not None:
                        add_dep_helper(ob.ins, prev.ins, False, "pool chain order")
                    prev = ob
                    pool_obs.append(ob)
                st = nc.gpsimd.dma_start(out=out_t[:, lo:hi], in_=o_sb[:, lo:hi])
                add_dep_helper(st.ins, prev.ins, False, "store after observers")
                stores.append(st)

            o_big = po.tile([P, 2048], mybir.dt.float32)  # 4 PSUM banks; matmul slices bank-aligned
            for g in range(2):
                ga = BIG * g
                if g == 1:
                    obL2 = nc.tensor.matmul(dummy[:], lhsT=xt_sb[:D_OUT, :D_OUT], rhs=xt_sb[:D_OUT, 0:1], start=True, stop=True)
                    add_dep_helper(obL2.ins, l2.ins, True, "PE observes load2")
                    add_dep_helper(obL2.ins, prev_pe.ins, False, "PE order")
                    prev_pe = obL2
                    obP = nc.tensor.matmul(dummy[:], lhsT=xt_sb[:D_OUT, :D_OUT], rhs=xt_sb[:D_OUT, 0:1], start=True, stop=True)
                    add_dep_helper(obP.ins, tts[0].ins, True, "PE observes round-0 release")
                    add_dep_helper(obP.ins, prev_pe.ins, False, "PE order")
                    prev_pe = obP
                    dvm2 = nc.vector.memset(scratch2[:, 2:3], 0.0)
                    add_dep_helper(dvm2.ins, lm2.ins, True, "DVE observes mask2")
                    add_dep_helper(dvm2.ins, prev_dve.ins, False, "DVE order")
                    prev_dve = dvm2

                for s0, s1 in ((0, 512), (512, 1024), (1024, 1536), (1536, BIG)):
                    for h in (0, 1):
                        rows = slice(D_OUT * h, D_OUT * (h + 1))
                        mm = nc.tensor.matmul(
                            o_big[rows, s0:s1],
                            lhsT=xt_sb[rows, :D_OUT],
                            rhs=xt_sb[rows, XOFF + ga + s0 : XOFF + ga + s1],
                            start=True, stop=True,
                        )
                        add_dep_helper(mm.ins, prev_pe.ins, False, "PE order")
                        prev_pe = mm
                        mms.append(mm)

                # one DVE observer + one full-width fused bias+mask op per round
                dob = nc.vector.memset(scratch2[:, 4 + g : 5 + g], 0.0)
                add_dep_helper(dob.ins, mms[-1].ins, True, "DVE observes round matmuls")
                add_dep_helper(dob.ins, prev_dve.ins, False, "DVE order")
                prev_dve = dob
                tt = nc.vector.scalar_tensor_tensor(
                    out=o_sb[:, ga : ga + BIG],
                    in0=o_big[:, :BIG],
                    scalar=xt_sb[:, D_OUT : D_OUT + 1],
                    in1=mask_sb[:, ga : ga + BIG],
                    op0=mybir.AluOpType.add,
                    op1=mybir.AluOpType.mult,
                )
                add_dep_helper(tt.ins, prev_dve.ins, False, "DVE order")
                prev_dve = tt
                tts.append(tt)
                emit_store(ga, ga + BIG, [(g, tt)])

            fin_pool = nc.gpsimd.memset(scratch[:, 19:20], 0.0)
            add_dep_helper(fin_pool.ins, stores[-1].ins, False, "after last store")

            chain = [l1, lm1, l2, lm2, stores[0], stores[1], tts[-1], mms[-1], fin_pool]
            chain_prev = None
            for dep in chain:
                nn = nc.sync.nop()
                add_dep_helper(nn.ins, dep.ins, True, "tail observe")
                add_dep_helper(nn.ins, stores[-1].ins, False, "tail after last store")
                if chain_prev is not None:
                    add_dep_helper(nn.ins, chain_prev.ins, False, "tail chain order")
                chain_prev = nn
    return nc


def _get_nc():
    if "nc" not in _cache:
        _cache["nc"] = _build()
    return _cache["nc"]


def _stage(node_feats, W_v, b_v, edge_index):
    nf_t = np.zeros((D_IN, VPAD), dtype=np.float32)
    nf_t[:, :V] = np.asarray(node_feats, dtype=np.float32).T
    Wv = np.asarray(W_v, np.float32)          # [64 in, 64 out] -> lhsT as-is
    bv = np.asarray(b_v, np.float32)
    dest = np.asarray(edge_index)[1].astype(np.int64)
    flag = np.zeros(VPAD, dtype=np.float32)
    flag[np.clip(dest, 0, V - 1)] = 1.0

    in_maps = []
    for c in range(NCORES):
        x_c = nf_t[:, SHARD * c : SHARD * (c + 1)]        # [64, 6272]
        f_c = flag[SHARD * c : SHARD * (c + 1)]           # [6272]
        xp = np.empty((P, XOFF + HALF), dtype=np.float32)
        mp = np.empty((P, HALF), dtype=ml_dtypes.bfloat16)
        for h in (0, 1):
            rows = slice(D_OUT * h, D_OUT * (h + 1))
            xp[rows, :D_OUT] = Wv
            xp[rows, D_OUT] = bv
            xp[rows, XOFF:] = x_c[:, HALF * h : HALF * (h + 1)]
            mp[rows, :] = f_c[None, HALF * h : HALF * (h + 1)].astype(ml_dtypes.bfloat16)
        in_maps.append({"xt": xp, "mb": mp})
    return in_maps


def _run(in_maps, **kwargs):
    nc = _get_nc()
    return run_bass_kernel_spmd(nc, in_maps, core_ids=list(range(NCORES)), **kwargs)


def kernel(
    node_feats, edge_feats, edge_index, W_q, b_q, W_k, b_k, W_v, b_v, W_e, b_e, a_w, a_b
) -> np.ndarray:
    in_maps = _stage(node_feats, W_v, b_v, edge_index)
    res = _run(in_maps)
    parts = []
    for c in range(NCORES):
        o = res.results[c]["out_t"]                       # [128, 3136]
        parts.append(np.concatenate([o[:D_OUT], o[D_OUT:]], axis=1))  # [64, 6272]
    full_t = np.concatenate(parts, axis=1)                # [64, 50176]
    return np.ascontiguousarray(full_t[:, :V].T).astype(np.float32)


# revision 29
# speedup vs baseline: 1.0786x; 1.0786x over previous
"""GATv2 layer kernel for 8 Trainium2 NeuronCores.

Mathematical structure exploited: in this GATv2 variant the value vectors are
gathered at the *destination* node (Vv = node_feats[dest] @ W_v + b_v), so for
every destination node d the aggregation

    out[d] = sum_{e: dest_e = d} alpha_e * (node_feats[d] @ W_v + b_v)
           = (node_feats[d] @ W_v + b_v) * sum_e alpha_e
           = (node_feats[d] @ W_v + b_v) * [deg_in(d) > 0]

because the softmax weights alpha sum to exactly 1 within each destination
segment (and the sum is empty for isolated nodes). Q/K/edge_feats/a_w only
reweight terms inside a softmax that cancels entirely. Verified against the
reference: max relative error ~2.6e-7 (pure fp32 rounding).

Device computation per core c (nodes sharded 6272/core): the node block is
split into two column halves stacked on the two 64-partition halves of SBUF
so every DMA runs at full 128-partition width. Per 392-column chunk and half:
one weight-stationary matmul (out.T = W_v.T @ x.T), then one fused DVE
scalar_tensor_tensor (add bias per output feature, multiply by the presence
mask). The mask ships as a pre-broadcast bf16 plane; W and b ship duplicated
per partition half inside the packed x tensor. The presence bitmap (50 KB)
is folded on the host during input sharding.

Sync-wait discipline: this container's walrus build allows only ONE semaphore
wait per instruction and Tile's sem assignment is not transitive, so the
stream is arranged so every op needs at most one new wait: a PE warm-up
matmul observes the first load, PE/DVE observers absorb the second-segment
loads, Pool memsets observe the DVE results before each SWDGE store, and a
final SP NoOp chain observes all async completions so the kernel-tail drain
needs no waits of its own.
"""
import ml_dtypes
import numpy as np

import concourse.bass as bass
import concourse.mybir as mybir
import concourse.tile as tile
from concourse.bass_utils import run_bass_kernel_spmd
from concourse.tile_rust import add_dep_helper

V, E = 50000, 800000
D_IN, D_OUT = 64, 64
NCORES = 8
P = 128
SHARD = 6272                # nodes per core
HALF = SHARD // 2           # 3136 columns per partition-half
VPAD = SHARD * NCORES       # 50176
MM = 392                    # node columns per matmul instruction
BIG = 1536                  # 3-PSUM-bank fused round
XOFF = D_OUT + 1            # x columns start after packed W (64) + b (1)
SPLIT = 1536                # segment boundary in node columns

_cache = {}


def _build():
    nc = bass.Bass()
    xt = nc.dram_tensor("xt", [P, XOFF + HALF], mybir.dt.float32, kind="ExternalInput")
    mb_d = nc.dram_tensor("mb", [P, HALF], mybir.dt.bfloat16, kind="ExternalInput")
    out_t = nc.dram_tensor("out_t", [P, HALF], mybir.dt.float32, kind="ExternalOutput")

    with tile.TileContext(nc) as tc:
        with (
            tc.tile_pool(name="const", bufs=1) as const,
            tc.tile_pool(name="po", bufs=2, space="PSUM") as po,
            tc.tile_pool(name="pd", bufs=1, space="PSUM") as pd,
        ):
            xt_sb = const.tile([P, XOFF + HALF], mybir.dt.float32)
            l1 = nc.sync.dma_start(out=xt_sb[:, : XOFF + SPLIT], in_=xt[:, : XOFF + SPLIT])
            mask_sb = const.tile([P, HALF], mybir.dt.bfloat16)
            lm1 = nc.sync.dma_start(out=mask_sb[:, :SPLIT], in_=mb_d[:, :SPLIT])
            l2 = nc.sync.dma_start(out=xt_sb[:, XOFF + SPLIT :], in_=xt[:, XOFF + SPLIT :])
            lm2 = nc.sync.dma_start(out=mask_sb[:, SPLIT:], in_=mb_d[:, SPLIT:])

            o_sb = const.tile([P, HALF], mybir.dt.float32)
            scratch = const.tile([1, 20], mybir.dt.float32)
            scratch2 = const.tile([1, 40], mybir.dt.float32)

            # PE warm-up observes load1; DVE observers for load1/mask1
            dummy = pd.tile([D_OUT, 1], mybir.dt.float32)
            mw = nc.tensor.matmul(dummy[:], lhsT=xt_sb[:D_OUT, :D_OUT], rhs=xt_sb[:D_OUT, 0:1], start=True, stop=True)
            add_dep_helper(mw.ins, l1.ins, True, "warm PE: observe load1")
            dvm0 = nc.vector.memset(scratch2[:, 0:1], 0.0)
            add_dep_helper(dvm0.ins, l1.ins, True, "DVE observes load1")
            dvm1 = nc.vector.memset(scratch2[:, 1:2], 0.0)
            add_dep_helper(dvm1.ins, lm1.ins, True, "DVE observes mask1")
            add_dep_helper(dvm1.ins, dvm0.ins, False, "DVE order")

            tts, mms, pool_obs, stores = [], [], [], []
            prev_pe = mw
            prev_dve = dvm1

            def emit_store(lo, hi, members):
                prev = pool_obs[-1] if pool_obs else None
                for k, t in members:
                    ob = nc.gpsimd.memset(scratch[:, k : k + 1], 0.0)
                    add_dep_helper(ob.ins, t.ins, True, "Pool observes TT")
                    if prev is not None:
                        add_dep_helper(ob.ins, prev.ins, False, "pool chain order")
                    prev = ob
                    pool_obs.append(ob)
                st = nc.gpsimd.dma_start(out=out_t[:, lo:hi], in_=o_sb[:, lo:hi])
                add_dep_helper(st.ins, prev.ins, False, "store after observers")
                stores.append(st)

            rounds = [(0, 1536), (1536, 3072), (3072, HALF)]
            for g, (ga, gb) in enumerate(rounds):
                o_big = po.tile([P, BIG], mybir.dt.float32, tag="obig")
                if g == 1:
                    obL2 = nc.tensor.matmul(dummy[:], lhsT=xt_sb[:D_OUT, :D_OUT], rhs=xt_sb[:D_OUT, 0:1], start=True, stop=True)
                    add_dep_helper(obL2.ins, l2.ins, True, "PE observes load2")
                    add_dep_helper(obL2.ins, prev_pe.ins, False, "PE order")
                    prev_pe = obL2
                    dvm2 = nc.vector.memset(scratch2[:, 2:3], 0.0)
                    add_dep_helper(dvm2.ins, lm2.ins, True, "DVE observes mask2")
                    add_dep_helper(dvm2.ins, prev_dve.ins, False, "DVE order")
                    prev_dve = dvm2

                if g == 2:
                    obP = nc.tensor.matmul(dummy[:], lhsT=xt_sb[:D_OUT, :D_OUT], rhs=xt_sb[:D_OUT, 0:1], start=True, stop=True)
                    add_dep_helper(obP.ins, tts[0].ins, True, "PE observes round-0 release")
                    add_dep_helper(obP.ins, prev_pe.ins, False, "PE order")
                    prev_pe = obP
                slices = [(c, min(c + 512, gb - ga)) for c in range(0, gb - ga, 512)]
                for s0, s1 in slices:
                    for h in (0, 1):
                        rows = slice(D_OUT * h, D_OUT * (h + 1))
                        mm = nc.tensor.matmul(
                            o_big[rows, s0:s1],
                            lhsT=xt_sb[rows, :D_OUT],
                            rhs=xt_sb[rows, XOFF + ga + s0 : XOFF + ga + s1],
                            start=True, stop=True,
                        )
                        add_dep_helper(mm.ins, prev_pe.ins, False, "PE order")
                        prev_pe = mm
                        mms.append(mm)

                # one DVE observer + one full-width fused bias+mask op per round
                dob = nc.vector.memset(scratch2[:, 4 + g : 5 + g], 0.0)
                add_dep_helper(dob.ins, mms[-1].ins, True, "DVE observes round matmuls")
                add_dep_helper(dob.ins, prev_dve.ins, False, "DVE order")
                prev_dve = dob
                tt = nc.vector.scalar_tensor_tensor(
                    out=o_sb[:, ga:gb],
                    in0=o_big[:, : gb - ga],
                    scalar=xt_sb[:, D_OUT : D_OUT + 1],
                    in1=mask_sb[:, ga:gb],
                    op0=mybir.AluOpType.add,
                    op1=mybir.AluOpType.mult,
                )
                add_dep_helper(tt.ins, prev_dve.ins, False, "DVE order")
                prev_dve = tt
                tts.append(tt)
                if g == 0:
                    emit_store(0, 1536, [(0, tt)])
                elif g == 2:
                    emit_store(1536, HALF, [(1, tts[1]), (2, tt)])

            fin_pool = nc.gpsimd.memset(scratch[:, 19:20], 0.0)
            add_dep_helper(fin_pool.ins, stores[-1].ins, False, "after last store")

            chain = [l1, lm1, l2, lm2, stores[0], stores[1], tts[-1], mms[-1], fin_pool]
            chain_prev = None
            for dep in chain:
                nn = nc.sync.nop()
                add_dep_helper(nn.ins, dep.ins, True, "tail observe")
                add_dep_helper(nn.ins, stores[-1].ins, False, "tail after last store")
                if chain_prev is not None:
                    add_dep_helper(nn.ins, chain_prev.ins, False, "tail chain order")
                chain_prev = nn
    return nc


def _get_nc():
    if "nc" not in _cache:
        _cache["nc"] = _build()
    return _cache["nc"]


def _stage(node_feats, W_v, b_v, edge_index):
    nf_t = np.zeros((D_IN, VPAD), dtype=np.float32)
    nf_t[:, :V] = np.asarray(node_feats, dtype=np.float32).T
    Wv = np.asarray(W_v, np.float32)          # [64 in, 64 out] -> lhsT as-is
    bv = np.asarray(b_v, np.float32)
    dest = np.asarray(edge_index)[1].astype(np.int64)
    flag = np.zeros(VPAD, dtype=np.float32)
    flag[np.clip(dest, 0, V - 1)] = 1.0

    in_maps = []
    for c in range(NCORES):
        x_c = nf_t[:, SHARD * c : SHARD * (c + 1)]        # [64, 6272]
        f_c = flag[SHARD * c : SHARD * (c + 1)]           # [6272]
        xp = np.empty((P, XOFF + HALF), dtype=np.float32)
        mp = np.empty((P, HALF), dtype=ml_dtypes.bfloat16)
        for h in (0, 1):
            rows = slice(D_OUT * h, D_OUT * (h + 1))
            xp[rows, :D_OUT] = Wv
            xp[rows, D_OUT] = bv
            xp[rows, XOFF:] = x_c[:, HALF * h : HALF * (h + 1)]
            mp[rows, :] = f_c[None, HALF * h : HALF * (h + 1)].astype(ml_dtypes.bfloat16)
        in_maps.append({"xt": xp, "mb": mp})
    return in_maps


def _run(in_maps, **kwargs):
    nc = _get_nc()
    return run_bass_kernel_spmd(nc, in_maps, core_ids=list(range(NCORES)), **kwargs)


def kernel(
    node_feats, edge_feats, edge_index, W_q, b_q, W_k, b_k, W_v, b_v, W_e, b_e, a_w, a_b
) -> np.ndarray:
    in_maps = _stage(node_feats, W_v, b_v, edge_index)
    res = _run(in_maps)
    parts = []
    for c in range(NCORES):
        o = res.results[c]["out_t"]                       # [128, 3136]
        parts.append(np.concatenate([o[:D_OUT], o[D_OUT:]], axis=1))  # [64, 6272]
    full_t = np.concatenate(parts, axis=1)                # [64, 50176]
    return np.ascontiguousarray(full_t[:, :V].T).astype(np.float32)


# revision 30
# speedup vs baseline: 1.1617x; 1.0770x over previous
"""GATv2 layer kernel for 8 Trainium2 NeuronCores.

Mathematical structure exploited: in this GATv2 variant the value vectors are
gathered at the *destination* node (Vv = node_feats[dest] @ W_v + b_v), so for
every destination node d the aggregation

    out[d] = sum_{e: dest_e = d} alpha_e * (node_feats[d] @ W_v + b_v)
           = (node_feats[d] @ W_v + b_v) * sum_e alpha_e
           = (node_feats[d] @ W_v + b_v) * [deg_in(d) > 0]

because the softmax weights alpha sum to exactly 1 within each destination
segment (and the sum is empty for isolated nodes). Q/K/edge_feats/a_w only
reweight terms inside a softmax that cancels entirely. Verified against the
reference: max relative error ~2.6e-7 (pure fp32 rounding).

Device computation per core c (nodes sharded 6272/core): the node block is
split into two column halves stacked on the two 64-partition halves of SBUF
so every DMA runs at full 128-partition width. Per 392-column chunk and half:
one weight-stationary matmul (out.T = W_v.T @ x.T), then one fused DVE
scalar_tensor_tensor (add bias per output feature, multiply by the presence
mask). The mask ships as a pre-broadcast bf16 plane; W and b ship duplicated
per partition half inside the packed x tensor. The presence bitmap (50 KB)
is folded on the host during input sharding.

Sync-wait discipline: this container's walrus build allows only ONE semaphore
wait per instruction and Tile's sem assignment is not transitive, so the
stream is arranged so every op needs at most one new wait: a PE warm-up
matmul observes the first load, PE/DVE observers absorb the second-segment
loads, Pool memsets observe the DVE results before each SWDGE store, and a
final SP NoOp chain observes all async completions so the kernel-tail drain
needs no waits of its own.
"""
import ml_dtypes
import numpy as np

import concourse.bass as bass
import concourse.mybir as mybir
import concourse.tile as tile
from concourse.bass_utils import run_bass_kernel_spmd
from concourse.tile_rust import add_dep_helper

V, E = 50000, 800000
D_IN, D_OUT = 64, 64
NCORES = 8
P = 128
SHARD = 6272                # nodes per core
HALF = SHARD // 2           # 3136 columns per partition-half
VPAD = SHARD * NCORES       # 50176
MM = 392                    # node columns per matmul chunk (8 per half)
NCH = HALF // MM            # 8
XOFF = D_OUT + 1            # x columns start after packed W (64) + b (1)
SPLIT = MM * 4              # segment boundary in node columns

_cache = {}


def _build():
    nc = bass.Bass()
    xt = nc.dram_tensor("xt", [P, XOFF + HALF], mybir.dt.float32, kind="ExternalInput")
    mb_d = nc.dram_tensor("mb", [P, HALF], mybir.dt.bfloat16, kind="ExternalInput")
    out_t = nc.dram_tensor("out_t", [P, HALF], mybir.dt.float32, kind="ExternalOutput")

    with tile.TileContext(nc) as tc:
        with (
            tc.tile_pool(name="const", bufs=1) as const,
            tc.tile_pool(name="po", bufs=3, space="PSUM") as po,
            tc.tile_pool(name="pd", bufs=1, space="PSUM") as pd,
        ):
            xt_sb = const.tile([P, XOFF + HALF], mybir.dt.float32)
            l1 = nc.sync.dma_start(out=xt_sb[:, : XOFF + SPLIT], in_=xt[:, : XOFF + SPLIT])
            mask_sb = const.tile([P, HALF], mybir.dt.bfloat16)
            lm1 = nc.sync.dma_start(out=mask_sb[:, :SPLIT], in_=mb_d[:, :SPLIT])
            l2 = nc.sync.dma_start(out=xt_sb[:, XOFF + SPLIT :], in_=xt[:, XOFF + SPLIT :])
            lm2 = nc.sync.dma_start(out=mask_sb[:, SPLIT:], in_=mb_d[:, SPLIT:])

            o_sb = const.tile([P, HALF], mybir.dt.float32)
            scratch = const.tile([1, 20], mybir.dt.float32)
            scratch2 = const.tile([1, 40], mybir.dt.float32)

            # PE warm-up observes load1; DVE observers for load1/mask1
            dummy = pd.tile([D_OUT, 1], mybir.dt.float32)
            mw = nc.tensor.matmul(dummy[:], lhsT=xt_sb[:D_OUT, :D_OUT], rhs=xt_sb[:D_OUT, 0:1], start=True, stop=True)
            add_dep_helper(mw.ins, l1.ins, True, "warm PE: observe load1")
            dvm0 = nc.vector.memset(scratch2[:, 0:1], 0.0)
            add_dep_helper(dvm0.ins, l1.ins, True, "DVE observes load1")
            dvm1 = nc.vector.memset(scratch2[:, 1:2], 0.0)
            add_dep_helper(dvm1.ins, lm1.ins, True, "DVE observes mask1")
            add_dep_helper(dvm1.ins, dvm0.ins, False, "DVE order")

            tts, mms, pool_obs, stores = [], [], [], []
            prev_pe = mw
            prev_dve = dvm1

            def emit_store(lo, hi, members):
                prev = pool_obs[-1] if pool_obs else None
                for k, t in members:
                    ob = nc.gpsimd.memset(scratch[:, k : k + 1], 0.0)
                    add_dep_helper(ob.ins, t.ins, True, "Pool observes TT")
                    if prev is not None:
                        add_dep_helper(ob.ins, prev.ins, False, "pool chain order")
                    prev = ob
                    pool_obs.append(ob)
                st = nc.gpsimd.dma_start(out=out_t[:, lo:hi], in_=o_sb[:, lo:hi])
                add_dep_helper(st.ins, prev.ins, False, "store after observers")
                stores.append(st)

            seg_members = []
            for j in range(NCH):
                a, b = MM * j, MM * (j + 1)
                if a == SPLIT:
                    obL2 = nc.tensor.matmul(dummy[:], lhsT=xt_sb[:D_OUT, :D_OUT], rhs=xt_sb[:D_OUT, 0:1], start=True, stop=True)
                    add_dep_helper(obL2.ins, l2.ins, True, "PE observes load2")
                    add_dep_helper(obL2.ins, prev_pe.ins, False, "PE order")
                    prev_pe = obL2
                    dvm2 = nc.vector.memset(scratch2[:, 2:3], 0.0)
                    add_dep_helper(dvm2.ins, lm2.ins, True, "DVE observes mask2")
                    add_dep_helper(dvm2.ins, prev_dve.ins, False, "DVE order")
                    prev_dve = dvm2

                o_pT = po.tile([P, MM], mybir.dt.float32, tag="opt")
                if j >= 3:
                    obP = nc.tensor.matmul(dummy[:], lhsT=xt_sb[:D_OUT, :D_OUT], rhs=xt_sb[:D_OUT, 0:1], start=True, stop=True)
                    add_dep_helper(obP.ins, tts[j - 3].ins, True, "PE observes slot release")
                    add_dep_helper(obP.ins, prev_pe.ins, False, "PE order")
                    prev_pe = obP
                for h in (0, 1):
                    rows = slice(D_OUT * h, D_OUT * (h + 1))
                    mm = nc.tensor.matmul(
                        o_pT[rows, :],
                        lhsT=xt_sb[rows, :D_OUT],
                        rhs=xt_sb[rows, XOFF + a : XOFF + b],
                        start=True, stop=True,
                    )
                    add_dep_helper(mm.ins, prev_pe.ins, False, "PE order")
                    prev_pe = mm
                    mms.append(mm)

                # one DVE observer + one full-width fused bias+mask op per chunk
                dob = nc.vector.memset(scratch2[:, 4 + j : 5 + j], 0.0)
                add_dep_helper(dob.ins, mms[-1].ins, True, "DVE observes matmul pair")
                add_dep_helper(dob.ins, prev_dve.ins, False, "DVE order")
                prev_dve = dob
                tt = nc.vector.scalar_tensor_tensor(
                    out=o_sb[:, a:b],
                    in0=o_pT[:],
                    scalar=xt_sb[:, D_OUT : D_OUT + 1],
                    in1=mask_sb[:, a:b],
                    op0=mybir.AluOpType.add,
                    op1=mybir.AluOpType.mult,
                )
                add_dep_helper(tt.ins, prev_dve.ins, False, "DVE order")
                prev_dve = tt
                tts.append(tt)
                seg_members.append((len(tts) - 1, tt))

                if b == SPLIT or b == HALF:
                    lo = 0 if b == SPLIT else SPLIT
                    emit_store(lo, b, seg_members)
                    seg_members = []

            fin_pool = nc.gpsimd.memset(scratch[:, 19:20], 0.0)
            add_dep_helper(fin_pool.ins, stores[-1].ins, False, "after last store")

            chain = [l1, lm1, l2, lm2, stores[0], stores[1], tts[-1], mms[-1], fin_pool]
            chain_prev = None
            for dep in chain:
                nn = nc.sync.nop()
                add_dep_helper(nn.ins, dep.ins, True, "tail observe")
                add_dep_helper(nn.ins, stores[-1].ins, False, "tail after last store")
                if chain_prev is not None:
                    add_dep_helper(nn.ins, chain_prev.ins, False, "tail chain order")
                chain_prev = nn
    return nc


def _get_nc():
    if "nc" not in _cache:
        _cache["nc"] = _build()
    return _cache["nc"]


def _stage(node_feats, W_v, b_v, edge_index):
    nf_t = np.zeros((D_IN, VPAD), dtype=np.float32)
    nf_t[:, :V] = np.asarray(node_feats, dtype=np.float32).T
    Wv = np.asarray(W_v, np.float32)          # [64 in, 64 out] -> lhsT as-is
    bv = np.asarray(b_v, np.float32)
    dest = np.asarray(edge_index)[1].astype(np.int64)
    flag = np.zeros(VPAD, dtype=np.float32)
    flag[np.clip(dest, 0, V - 1)] = 1.0

    in_maps = []
    for c in range(NCORES):
        x_c = nf_t[:, SHARD * c : SHARD * (c + 1)]        # [64, 6272]
        f_c = flag[SHARD * c : SHARD * (c + 1)]           # [6272]
        xp = np.empty((P, XOFF + HALF), dtype=np.float32)
        mp = np.empty((P, HALF), dtype=ml_dtypes.bfloat16)
        for h in (0, 1):
            rows = slice(D_OUT * h, D_OUT * (h + 1))
            xp[rows, :D_OUT] = Wv
            xp[rows, D_OUT] = bv
            xp[rows, XOFF:] = x_c[:, HALF * h : HALF * (h + 1)]
            mp[rows, :] = f_c[None, HALF * h : HALF * (h + 1)].astype(ml_dtypes.bfloat16)
        in_maps.append({"xt": xp, "mb": mp})
    return in_maps


def _run(in_maps, **kwargs):
    nc = _get_nc()
    return run_bass_kernel_spmd(nc, in_maps, core_ids=list(range(NCORES)), **kwargs)


def kernel(
    node_feats, edge_feats, edge_index, W_q, b_q, W_k, b_k, W_v, b_v, W_e, b_e, a_w, a_b
) -> np.ndarray:
    in_maps = _stage(node_feats, W_v, b_v, edge_index)
    res = _run(in_maps)
    parts = []
    for c in range(NCORES):
        o = res.results[c]["out_t"]                       # [128, 3136]
        parts.append(np.concatenate([o[:D_OUT], o[D_OUT:]], axis=1))  # [64, 6272]
    full_t = np.concatenate(parts, axis=1)                # [64, 50176]
    return np.ascontiguousarray(full_t[:, :V].T).astype(np.float32)


# revision 32
# speedup vs baseline: 1.2278x; 1.0569x over previous
"""GATv2 layer kernel for 8 Trainium2 NeuronCores.

Mathematical structure exploited: in this GATv2 variant the value vectors are
gathered at the *destination* node (Vv = node_feats[dest] @ W_v + b_v), so for
every destination node d the aggregation

    out[d] = sum_{e: dest_e = d} alpha_e * (node_feats[d] @ W_v + b_v)
           = (node_feats[d] @ W_v + b_v) * sum_e alpha_e
           = (node_feats[d] @ W_v + b_v) * [deg_in(d) > 0]

because the softmax weights alpha sum to exactly 1 within each destination
segment (and the sum is empty for isolated nodes). Q/K/edge_feats/a_w only
reweight terms inside a softmax that cancels entirely. Verified against the
reference: max relative error ~2.6e-7 (pure fp32 rounding).

Device computation per core c (nodes sharded 6272/core): the node block is
split into two column halves stacked on the two 64-partition halves of SBUF
so every DMA runs at full 128-partition width. Per 392-column chunk and half:
one weight-stationary matmul (out.T = W_v.T @ x.T), then one fused DVE
scalar_tensor_tensor (add bias per output feature, multiply by the presence
mask). The mask ships as a pre-broadcast bf16 plane; W and b ship duplicated
per partition half inside the packed x tensor. The presence bitmap (50 KB)
is folded on the host during input sharding.

Sync-wait discipline: this container's walrus build allows only ONE semaphore
wait per instruction and Tile's sem assignment is not transitive, so the
stream is arranged so every op needs at most one new wait: a PE warm-up
matmul observes the first load, PE/DVE observers absorb the second-segment
loads, Pool memsets observe the DVE results before each SWDGE store, and a
final SP NoOp chain observes all async completions so the kernel-tail drain
needs no waits of its own.
"""
import ml_dtypes
import numpy as np

import concourse.bass as bass
import concourse.mybir as mybir
import concourse.tile as tile
from concourse.bass_utils import run_bass_kernel_spmd
from concourse.tile_rust import add_dep_helper

V, E = 50000, 800000
D_IN, D_OUT = 64, 64
NCORES = 8
P = 128
SHARD = 6272                # nodes per core
HALF = SHARD // 2           # 3136 columns per partition-half
VPAD = SHARD * NCORES       # 50176
MM = 392                    # node columns per matmul chunk (8 per half)
NCH = HALF // MM            # 8
XOFF = D_OUT + 1            # x columns start after packed W (64) + b (1)
SPLIT = MM * 4              # segment boundary in node columns

_cache = {}


def _build():
    nc = bass.Bass()
    xt = nc.dram_tensor("xt", [P, XOFF + HALF], mybir.dt.float32, kind="ExternalInput")
    mb_d = nc.dram_tensor("mb", [P, HALF], mybir.dt.bfloat16, kind="ExternalInput")
    out_t = nc.dram_tensor("out_t", [P, HALF], mybir.dt.float32, kind="ExternalOutput")

    with tile.TileContext(nc) as tc:
        with (
            tc.tile_pool(name="const", bufs=1) as const,
            tc.tile_pool(name="po", bufs=4, space="PSUM") as po,
            tc.tile_pool(name="pd", bufs=1, space="PSUM") as pd,
        ):
            xt_sb = const.tile([P, XOFF + HALF], mybir.dt.float32)
            l1 = nc.sync.dma_start(out=xt_sb[:, : XOFF + SPLIT], in_=xt[:, : XOFF + SPLIT])
            mask_sb = const.tile([P, HALF], mybir.dt.bfloat16)
            lm1 = nc.sync.dma_start(out=mask_sb[:, :SPLIT], in_=mb_d[:, :SPLIT])
            l2 = nc.sync.dma_start(out=xt_sb[:, XOFF + SPLIT :], in_=xt[:, XOFF + SPLIT :])
            lm2 = nc.sync.dma_start(out=mask_sb[:, SPLIT:], in_=mb_d[:, SPLIT:])

            o_sb = const.tile([P, HALF], mybir.dt.float32)
            scratch = const.tile([1, 20], mybir.dt.float32)
            scratch2 = const.tile([1, 40], mybir.dt.float32)

            # PE warm-up observes load1; DVE observers for load1/mask1
            dummy = pd.tile([D_OUT, 1], mybir.dt.float32)
            mw = nc.tensor.matmul(dummy[:], lhsT=xt_sb[:D_OUT, :D_OUT], rhs=xt_sb[:D_OUT, 0:1], start=True, stop=True)
            add_dep_helper(mw.ins, l1.ins, True, "warm PE: observe load1")
            dvm0 = nc.vector.memset(scratch2[:, 0:1], 0.0)
            add_dep_helper(dvm0.ins, l1.ins, True, "DVE observes load1")
            dvm1 = nc.vector.memset(scratch2[:, 1:2], 0.0)
            add_dep_helper(dvm1.ins, lm1.ins, True, "DVE observes mask1")
            add_dep_helper(dvm1.ins, dvm0.ins, False, "DVE order")

            tts, mms, pool_obs, stores = [], [], [], []
            prev_pe = mw
            prev_dve = dvm1

            def emit_store(lo, hi, members):
                prev = pool_obs[-1] if pool_obs else None
                for k, t in members:
                    ob = nc.gpsimd.memset(scratch[:, k : k + 1], 0.0)
                    add_dep_helper(ob.ins, t.ins, True, "Pool observes TT")
                    if prev is not None:
                        add_dep_helper(ob.ins, prev.ins, False, "pool chain order")
                    prev = ob
                    pool_obs.append(ob)
                st = nc.gpsimd.dma_start(out=out_t[:, lo:hi], in_=o_sb[:, lo:hi])
                add_dep_helper(st.ins, prev.ins, False, "store after observers")
                stores.append(st)

            seg_members = []
            for j in range(NCH):
                a, b = MM * j, MM * (j + 1)
                if a == SPLIT:
                    obL2 = nc.tensor.matmul(dummy[:], lhsT=xt_sb[:D_OUT, :D_OUT], rhs=xt_sb[:D_OUT, 0:1], start=True, stop=True)
                    add_dep_helper(obL2.ins, l2.ins, True, "PE observes load2")
                    add_dep_helper(obL2.ins, prev_pe.ins, False, "PE order")
                    prev_pe = obL2
                    dvm2 = nc.vector.memset(scratch2[:, 2:3], 0.0)
                    add_dep_helper(dvm2.ins, lm2.ins, True, "DVE observes mask2")
                    add_dep_helper(dvm2.ins, prev_dve.ins, False, "DVE order")
                    prev_dve = dvm2

                o_pT = po.tile([P, MM], mybir.dt.float32, tag="opt")
                if j >= 4:
                    obP = nc.tensor.matmul(dummy[:], lhsT=xt_sb[:D_OUT, :D_OUT], rhs=xt_sb[:D_OUT, 0:1], start=True, stop=True)
                    add_dep_helper(obP.ins, tts[j - 4].ins, True, "PE observes slot release")
                    add_dep_helper(obP.ins, prev_pe.ins, False, "PE order")
                    prev_pe = obP
                for h in (0, 1):
                    rows = slice(D_OUT * h, D_OUT * (h + 1))
                    mm = nc.tensor.matmul(
                        o_pT[rows, :],
                        lhsT=xt_sb[rows, :D_OUT],
                        rhs=xt_sb[rows, XOFF + a : XOFF + b],
                        start=True, stop=True,
                    )
                    add_dep_helper(mm.ins, prev_pe.ins, False, "PE order")
                    prev_pe = mm
                    mms.append(mm)

                # one DVE observer + one full-width fused bias+mask op per chunk
                dob = nc.vector.memset(scratch2[:, 4 + j : 5 + j], 0.0)
                add_dep_helper(dob.ins, mms[-1].ins, True, "DVE observes matmul pair")
                add_dep_helper(dob.ins, prev_dve.ins, False, "DVE order")
                prev_dve = dob
                tt = nc.vector.scalar_tensor_tensor(
                    out=o_sb[:, a:b],
                    in0=o_pT[:],
                    scalar=xt_sb[:, D_OUT : D_OUT + 1],
                    in1=mask_sb[:, a:b],
                    op0=mybir.AluOpType.add,
                    op1=mybir.AluOpType.mult,
                )
                add_dep_helper(tt.ins, prev_dve.ins, False, "DVE order")
                prev_dve = tt
                tts.append(tt)
                seg_members.append((len(tts) - 1, tt))

                if (j + 1) % 2 == 0:
                    emit_store(b - 2 * MM, b, seg_members)
                    seg_members = []

            fin_pool = nc.gpsimd.memset(scratch[:, 19:20], 0.0)
            add_dep_helper(fin_pool.ins, stores[-1].ins, False, "after last store")

            chain = [l1, lm1, l2, lm2] + stores + [tts[-1], mms[-1], fin_pool]
            chain_prev = None
            for dep in chain:
                nn = nc.sync.nop()
                add_dep_helper(nn.ins, dep.ins, True, "tail observe")
                add_dep_helper(nn.ins, stores[-1].ins, False, "tail after last store")
                if chain_prev is not None:
                    add_dep_helper(nn.ins, chain_prev.ins, False, "tail chain order")
                chain_prev = nn
    return nc


def _get_nc():
    if "nc" not in _cache:
        _cache["nc"] = _build()
    return _cache["nc"]


def _stage(node_feats, W_v, b_v, edge_index):
    nf_t = np.zeros((D_IN, VPAD), dtype=np.float32)
    nf_t[:, :V] = np.asarray(node_feats, dtype=np.float32).T
    Wv = np.asarray(W_v, np.float32)          # [64 in, 64 out] -> lhsT as-is
    bv = np.asarray(b_v, np.float32)
    dest = np.asarray(edge_index)[1].astype(np.int64)
    flag = np.zeros(VPAD, dtype=np.float32)
    flag[np.clip(dest, 0, V - 1)] = 1.0

    in_maps = []
    for c in range(NCORES):
        x_c = nf_t[:, SHARD * c : SHARD * (c + 1)]        # [64, 6272]
        f_c = flag[SHARD * c : SHARD * (c + 1)]           # [6272]
        xp = np.empty((P, XOFF + HALF), dtype=np.float32)
        mp = np.empty((P, HALF), dtype=ml_dtypes.bfloat16)
        for h in (0, 1):
            rows = slice(D_OUT * h, D_OUT * (h + 1))
            xp[rows, :D_OUT] = Wv
            xp[rows, D_OUT] = bv
            xp[rows, XOFF:] = x_c[:, HALF * h : HALF * (h + 1)]
            mp[rows, :] = f_c[None, HALF * h : HALF * (h + 1)].astype(ml_dtypes.bfloat16)
        in_maps.append({"xt": xp, "mb": mp})
    return in_maps


def _run(in_maps, **kwargs):
    nc = _get_nc()
    return run_bass_kernel_spmd(nc, in_maps, core_ids=list(range(NCORES)), **kwargs)


def kernel(
    node_feats, edge_feats, edge_index, W_q, b_q, W_k, b_k, W_v, b_v, W_e, b_e, a_w, a_b
) -> np.ndarray:
    in_maps = _stage(node_feats, W_v, b_v, edge_index)
    res = _run(in_maps)
    parts = []
    for c in range(NCORES):
        o = res.results[c]["out_t"]                       # [128, 3136]
        parts.append(np.concatenate([o[:D_OUT], o[D_OUT:]], axis=1))  # [64, 6272]
    full_t = np.concatenate(parts, axis=1)                # [64, 50176]
    return np.ascontiguousarray(full_t[:, :V].T).astype(np.float32)


# revision 33
# speedup vs baseline: 1.2281x; 1.0003x over previous
"""GATv2 layer kernel for 8 Trainium2 NeuronCores.

Mathematical structure exploited: in this GATv2 variant the value vectors are
gathered at the *destination* node (Vv = node_feats[dest] @ W_v + b_v), so for
every destination node d the aggregation

    out[d] = sum_{e: dest_e = d} alpha_e * (node_feats[d] @ W_v + b_v)
           = (node_feats[d] @ W_v + b_v) * sum_e alpha_e
           = (node_feats[d] @ W_v + b_v) * [deg_in(d) > 0]

because the softmax weights alpha sum to exactly 1 within each destination
segment (and the sum is empty for isolated nodes). Q/K/edge_feats/a_w only
reweight terms inside a softmax that cancels entirely. Verified against the
reference: max relative error ~2.6e-7 (pure fp32 rounding).

Device computation per core c (nodes sharded 6272/core): the node block is
split into two column halves stacked on the two 64-partition halves of SBUF
so every DMA runs at full 128-partition width. Per 392-column chunk and half:
one weight-stationary matmul (out.T = W_v.T @ x.T), then one fused DVE
scalar_tensor_tensor (add bias per output feature, multiply by the presence
mask). The mask ships as a pre-broadcast bf16 plane; W and b ship duplicated
per partition half inside the packed x tensor. The presence bitmap (50 KB)
is folded on the host during input sharding.

Sync-wait discipline: this container's walrus build allows only ONE semaphore
wait per instruction and Tile's sem assignment is not transitive, so the
stream is arranged so every op needs at most one new wait: a PE warm-up
matmul observes the first load, PE/DVE observers absorb the second-segment
loads, Pool memsets observe the DVE results before each SWDGE store, and a
final SP NoOp chain observes all async completions so the kernel-tail drain
needs no waits of its own.
"""
import ml_dtypes
import numpy as np

import concourse.bass as bass
import concourse.mybir as mybir
import concourse.tile as tile
from concourse.bass_utils import run_bass_kernel_spmd
from concourse.tile_rust import add_dep_helper

V, E = 50000, 800000
D_IN, D_OUT = 64, 64
NCORES = 8
P = 128
SHARD = 6272                # nodes per core
HALF = SHARD // 2           # 3136 columns per partition-half
VPAD = SHARD * NCORES       # 50176
MM = 392                    # node columns per matmul chunk (8 per half)
NCH = HALF // MM            # 8
XOFF = D_OUT + 1            # x columns start after packed W (64) + b (1)
SPLIT = MM * 4              # segment boundary in node columns

_cache = {}


def _build():
    nc = bass.Bass()
    xt = nc.dram_tensor("xt", [P, XOFF + HALF], mybir.dt.float32, kind="ExternalInput")
    mb_d = nc.dram_tensor("mb", [P, HALF], mybir.dt.bfloat16, kind="ExternalInput")
    out_t = nc.dram_tensor("out_t", [P, HALF], mybir.dt.float32, kind="ExternalOutput")

    with tile.TileContext(nc) as tc:
        with (
            tc.tile_pool(name="const", bufs=1) as const,
            tc.tile_pool(name="po", bufs=7, space="PSUM") as po,
            tc.tile_pool(name="pd", bufs=1, space="PSUM") as pd,
        ):
            xt_sb = const.tile([P, XOFF + HALF], mybir.dt.float32)
            l1 = nc.sync.dma_start(out=xt_sb[:, : XOFF + SPLIT], in_=xt[:, : XOFF + SPLIT])
            mask_sb = const.tile([P, HALF], mybir.dt.bfloat16)
            lm1 = nc.sync.dma_start(out=mask_sb[:, :SPLIT], in_=mb_d[:, :SPLIT])
            l2 = nc.sync.dma_start(out=xt_sb[:, XOFF + SPLIT :], in_=xt[:, XOFF + SPLIT :])
            lm2 = nc.sync.dma_start(out=mask_sb[:, SPLIT:], in_=mb_d[:, SPLIT:])

            o_sb = const.tile([P, HALF], mybir.dt.float32)
            scratch = const.tile([1, 20], mybir.dt.float32)
            scratch2 = const.tile([1, 40], mybir.dt.float32)

            # PE warm-up observes load1; DVE observers for load1/mask1
            dummy = pd.tile([D_OUT, 1], mybir.dt.float32)
            mw = nc.tensor.matmul(dummy[:], lhsT=xt_sb[:D_OUT, :D_OUT], rhs=xt_sb[:D_OUT, 0:1], start=True, stop=True)
            add_dep_helper(mw.ins, l1.ins, True, "warm PE: observe load1")
            dvm0 = nc.vector.memset(scratch2[:, 0:1], 0.0)
            add_dep_helper(dvm0.ins, l1.ins, True, "DVE observes load1")
            dvm1 = nc.vector.memset(scratch2[:, 1:2], 0.0)
            add_dep_helper(dvm1.ins, lm1.ins, True, "DVE observes mask1")
            add_dep_helper(dvm1.ins, dvm0.ins, False, "DVE order")

            tts, mms, pool_obs, stores = [], [], [], []
            prev_pe = mw
            prev_dve = dvm1

            def emit_store(lo, hi, members):
                prev = pool_obs[-1] if pool_obs else None
                for k, t in members:
                    ob = nc.gpsimd.memset(scratch[:, k : k + 1], 0.0)
                    add_dep_helper(ob.ins, t.ins, True, "Pool observes TT")
                    if prev is not None:
                        add_dep_helper(ob.ins, prev.ins, False, "pool chain order")
                    prev = ob
                    pool_obs.append(ob)
                st = nc.gpsimd.dma_start(out=out_t[:, lo:hi], in_=o_sb[:, lo:hi])
                add_dep_helper(st.ins, prev.ins, False, "store after observers")
                stores.append(st)

            seg_members = []
            for j in range(NCH):
                a, b = MM * j, MM * (j + 1)
                if a == SPLIT:
                    obL2 = nc.tensor.matmul(dummy[:], lhsT=xt_sb[:D_OUT, :D_OUT], rhs=xt_sb[:D_OUT, 0:1], start=True, stop=True)
                    add_dep_helper(obL2.ins, l2.ins, True, "PE observes load2")
                    add_dep_helper(obL2.ins, prev_pe.ins, False, "PE order")
                    prev_pe = obL2
                    dvm2 = nc.vector.memset(scratch2[:, 2:3], 0.0)
                    add_dep_helper(dvm2.ins, lm2.ins, True, "DVE observes mask2")
                    add_dep_helper(dvm2.ins, prev_dve.ins, False, "DVE order")
                    prev_dve = dvm2

                o_pT = po.tile([P, MM], mybir.dt.float32, tag="opt")
                if j >= 7:
                    obP = nc.tensor.matmul(dummy[:], lhsT=xt_sb[:D_OUT, :D_OUT], rhs=xt_sb[:D_OUT, 0:1], start=True, stop=True)
                    add_dep_helper(obP.ins, tts[j - 7].ins, True, "PE observes slot release")
                    add_dep_helper(obP.ins, prev_pe.ins, False, "PE order")
                    prev_pe = obP
                for h in (0, 1):
                    rows = slice(D_OUT * h, D_OUT * (h + 1))
                    mm = nc.tensor.matmul(
                        o_pT[rows, :],
                        lhsT=xt_sb[rows, :D_OUT],
                        rhs=xt_sb[rows, XOFF + a : XOFF + b],
                        start=True, stop=True,
                    )
                    add_dep_helper(mm.ins, prev_pe.ins, False, "PE order")
                    prev_pe = mm
                    mms.append(mm)

                # one DVE observer + one full-width fused bias+mask op per chunk
                dob = nc.vector.memset(scratch2[:, 4 + j : 5 + j], 0.0)
                add_dep_helper(dob.ins, mms[-1].ins, True, "DVE observes matmul pair")
                add_dep_helper(dob.ins, prev_dve.ins, False, "DVE order")
                prev_dve = dob
                tt = nc.vector.scalar_tensor_tensor(
                    out=o_sb[:, a:b],
                    in0=o_pT[:],
                    scalar=xt_sb[:, D_OUT : D_OUT + 1],
                    in1=mask_sb[:, a:b],
                    op0=mybir.AluOpType.add,
                    op1=mybir.AluOpType.mult,
                )
                add_dep_helper(tt.ins, prev_dve.ins, False, "DVE order")
                prev_dve = tt
                tts.append(tt)
                seg_members.append((len(tts) - 1, tt))

                if (j + 1) % 2 == 0:
                    emit_store(b - 2 * MM, b, seg_members)
                    seg_members = []

            fin_pool = nc.gpsimd.memset(scratch[:, 19:20], 0.0)
            add_dep_helper(fin_pool.ins, stores[-1].ins, False, "after last store")

            chain = [l1, lm1, l2, lm2] + stores + [tts[-1], mms[-1], fin_pool]
            chain_prev = None
            for dep in chain:
                nn = nc.sync.nop()
                add_dep_helper(nn.ins, dep.ins, True, "tail observe")
                add_dep_helper(nn.ins, stores[-1].ins, False, "tail after last store")
                if chain_prev is not None:
                    add_dep_helper(nn.ins, chain_prev.ins, False, "tail chain order")
                chain_prev = nn
    return nc


def _get_nc():
    if "nc" not in _cache:
        _cache["nc"] = _build()
    return _cache["nc"]


def _stage(node_feats, W_v, b_v, edge_index):
    nf_t = np.zeros((D_IN, VPAD), dtype=np.float32)
    nf_t[:, :V] = np.asarray(node_feats, dtype=np.float32).T
    Wv = np.asarray(W_v, np.float32)          # [64 in, 64 out] -> lhsT as-is
    bv = np.asarray(b_v, np.float32)
    dest = np.asarray(edge_index)[1].astype(np.int64)
    flag = np.zeros(VPAD, dtype=np.float32)
    flag[np.clip(dest, 0, V - 1)] = 1.0

    in_maps = []
    for c in range(NCORES):
        x_c = nf_t[:, SHARD * c : SHARD * (c + 1)]        # [64, 6272]
        f_c = flag[SHARD * c : SHARD * (c + 1)]           # [6272]
        xp = np.empty((P, XOFF + HALF), dtype=np.float32)
        mp = np.empty((P, HALF), dtype=ml_dtypes.bfloat16)
        for h in (0, 1):
            rows = slice(D_OUT * h, D_OUT * (h + 1))
            xp[rows, :D_OUT] = Wv
            xp[rows, D_OUT] = bv
            xp[rows, XOFF:] = x_c[:, HALF * h : HALF * (h + 1)]
            mp[rows, :] = f_c[None, HALF * h : HALF * (h + 1)].astype(ml_dtypes.bfloat16)
        in_maps.append({"xt": xp, "mb": mp})
    return in_maps


def _run(in_maps, **kwargs):
    nc = _get_nc()
    return run_bass_kernel_spmd(nc, in_maps, core_ids=list(range(NCORES)), **kwargs)


def kernel(
    node_feats, edge_feats, edge_index, W_q, b_q, W_k, b_k, W_v, b_v, W_e, b_e, a_w, a_b
) -> np.ndarray:
    in_maps = _stage(node_feats, W_v, b_v, edge_index)
    res = _run(in_maps)
    parts = []
    for c in range(NCORES):
        o = res.results[c]["out_t"]                       # [128, 3136]
        parts.append(np.concatenate([o[:D_OUT], o[D_OUT:]], axis=1))  # [64, 6272]
    full_t = np.concatenate(parts, axis=1)                # [64, 50176]
    return np.ascontiguousarray(full_t[:, :V].T).astype(np.float32)
